# revision 30
# baseline (speedup 1.0000x reference)
"""Trainium2 Bass kernel for nn_Block_7584912244953 (gnn_message_passing).

Strategy (8 NeuronCores, SPMD, node-row sharding, no collectives):
  - Associativity: Wp1 @ (e @ W) == (Wp1 @ e) @ W. Each core computes
    T_b^T = e^T @ Wp1_b[rows]^T for its 512 node rows (contraction over
    all 16384 edges), then small (512x512) projections.
  - RMS norms folded on host; gains and 1/sqrt(D) folded into weights.
  - Schedule: T_hh pass -> node-side projections -> T_he pass (sdpa_hh
    on DVE underneath) -> T_eh pass (sdpa_he underneath) -> T_ee pass
    (sdpa_eh underneath) -> sdpa_ee + FFN (software-pipelined).
  - Stream DMAs batched 4 m-tiles per transfer (HWDGE issue rate is
    ~625ns/DMA; fewer, bigger transfers).
  - SDPA on DVE with broadcast APs, all-bf16 operands (2x mode); exp
    values scaled by 1/den before the AV product so no final rescale.
"""

import numpy as np
import ml_dtypes

BF16 = ml_dtypes.bfloat16
NCORES = 8
H, D = 8, 64
G = 4  # m-tiles per DMA chunk
_CACHE = {}


def _dims(scale=1):
    N, M, E = 4096 // scale, 16384 // scale, 512
    R = N // NCORES
    return dict(N=N, M=M, E=E, R=R, NT=R // 128, ET=E // 128, MT=M // 128,
                NMT=N // 128, F=4 * E, FT=4 * E // 128,
                MC=M // 128 // G, NC=N // 128 // G)


WT_B = ["q_hh", "q_ee", "k_ee", "q_eh", "k_he"]
WN_B = ["k_hh", "v_hh", "v_ee", "k_eh", "v_eh", "q_he", "v_he"]


def _build(scale=1, loopn=1, sim_safe=False):
    import concourse.bacc as bacc
    import concourse.mybir as mybir
    from concourse import tile

    dm = _dims(scale)
    N, M, E, R = dm["N"], dm["M"], dm["E"], dm["R"]
    NT, ET, F, FT = dm["NT"], dm["ET"], dm["F"], dm["FT"]
    MC, NC = dm["MC"], dm["NC"]

    F32 = mybir.dt.float32
    F32R = mybir.dt.float32r
    B16 = mybir.dt.bfloat16
    F8 = mybir.dt.float8e4
    AF = mybir.ActivationFunctionType
    ALU = mybir.AluOpType
    AX = mybir.AxisListType

    nc = bacc.Bacc("TRN2", target_bir_lowering=False, debug=False, num_devices=NCORES)

    d_xe = nc.dram_tensor("xe", [MC * 128, G * E], B16, kind="ExternalInput")
    d_wp = {b: nc.dram_tensor(f"wp1t_{b}", [MC * 128, G * R], B16, kind="ExternalInput")
            for b in ("ee", "eh", "he")}
    d_adjt = nc.dram_tensor("adjt", [NC * 128, G * R], F8, kind="ExternalInput")
    d_xnb = nc.dram_tensor("xnb", [NC * 128, G * E], B16, kind="ExternalInput")
    d_xnt = nc.dram_tensor("xnt", [128, ET * R], B16, kind="ExternalInput")
    d_wt = {w: nc.dram_tensor(f"w_{w}", [128, ET * E], B16, kind="ExternalInput") for w in WT_B}
    d_wn = {w: nc.dram_tensor(f"w_{w}", [128, ET * E], B16, kind="ExternalInput") for w in WN_B}
    d_wf1 = nc.dram_tensor("wf1", [FT * 128, ET * 128], B16, kind="ExternalInput")
    d_b1t = nc.dram_tensor("b1t", [128, FT], F32, kind="ExternalInput")
    d_wf2 = nc.dram_tensor("wf2", [F, E], B16, kind="ExternalInput")
    d_wtb = nc.dram_tensor("wtb", [128, E], B16, kind="ExternalInput")
    d_id = nc.dram_tensor("ident", [128, 128], B16, kind="ExternalInput")
    d_ones = nc.dram_tensor("onesrow", [1, 128], B16, kind="ExternalInput")
    d_out = nc.dram_tensor("out", [R, E], F32, kind="ExternalOutput")

    with tile.TileContext(nc) as tc:
        with (
            tc.tile_pool(name="stream", bufs=5) as st,
            tc.tile_pool(name="xnts", bufs=1) as xs,
            tc.tile_pool(name="wtres", bufs=1) as wtp,
            tc.tile_pool(name="wnres", bufs=1) as wn_,
            tc.tile_pool(name="tstore", bufs=1) as ts_,
            tc.tile_pool(name="qkv", bufs=1) as qs,
            tc.tile_pool(name="sdpa1", bufs=2) as sp1,
            tc.tile_pool(name="sdpa2", bufs=1) as sp2,
            tc.tile_pool(name="sdpa", bufs=2) as sp,
            tc.tile_pool(name="xacc", bufs=1) as xa,
            tc.tile_pool(name="zst", bufs=2) as zs,
            tc.tile_pool(name="wstream", bufs=2) as ws,
            tc.tile_pool(name="misc", bufs=1) as mp,
            tc.tile_pool(name="psum", bufs=1, space="PSUM") as pp,
        ):
            def body(iv=None):
                qkv = {}     # (name, t) -> sbuf bf16 tile (128, 512)
                tstore = {}  # (branch, e) -> T^T tile bf16 (128, R)

                # ---- residents (loaded lazily, after the first stream chunks)
                xnt_c = xs.tile([128, ET * R], B16, tag="xntc")
                ident = mp.tile([128, 128], B16, tag="ident")
                eps_t = mp.tile([128, 1], F32, tag="eps")

                def load_residents():
                    nc.sync.dma_start(out=xnt_c[:], in_=d_xnt.ap())
                    nc.sync.dma_start(out=ident[:], in_=d_id.ap())
                    nc.gpsimd.memset(eps_t[:], 1e-6)

                _wn_slot = [0]

                def load_wn(name):
                    s = _wn_slot[0] = (_wn_slot[0] + 1) % 3
                    t_ = wn_.tile([128, ET * E], B16, tag=f"wn_{s}")
                    nc.sync.dma_start(out=t_[:], in_=d_wn[name].ap())
                    return lambda k: t_[:, k * E:(k + 1) * E]

                _wt_slot = [0]

                def wt_ap(name):
                    s = _wt_slot[0] = (_wt_slot[0] + 1) % 2
                    t_ = wtp.tile([128, ET * E], B16, tag=f"wt_{s}", name=f"wt_{s}")
                    nc.sync.dma_start(out=t_[:], in_=d_wt[name].ap())
                    return lambda k: t_[:, k * E:(k + 1) * E]

                def xnt_src(k, t):
                    return xnt_c[:, k * R + t * 128: k * R + (t + 1) * 128]

                def ts_src(branch):
                    return lambda k, t: tstore[(branch, k)][:, t * 128:(t + 1) * 128]

                # ---- grouped projections: (name, src_fn(k,t), wt_fn(k))
                def proj_group(specs, banks):
                    for t in range(NT):
                        psb = [pp.tile([128, E], F32, tag=f"bank{banks[i]}",
                                       name=f"psb{banks[i]}")
                               for i in range(len(specs))]
                        for k in range(ET):
                            for i, (name, src, wtf) in enumerate(specs):
                                nc.tensor.matmul(psb[i][:], src(k, t), wtf(k),
                                                 start=(k == 0), stop=(k == ET - 1))
                        for i, (name, src, wtf) in enumerate(specs):
                            q = qs.tile([128, E], B16, tag=f"{name}{t}", name=f"{name}{t}")
                            nc.scalar.copy(q[:], psb[i][:])
                            qkv[(name, t)] = q

                # ---- T passes: contraction over chunked streams
                def edge_pass(branches, banks, d_x, d_w, nchunks, width,
                                              wdt=B16, after_chunk=None):
                    ps = {b: [pp.tile([128, R], F32, tag=f"bank{banks[b][e]}",
                                      name=f"ps_{b}{e}")
                              for e in range(ET)] for b in branches}
                    for c in range(nchunks):
                        xt_ = st.tile([128, G * width], B16, tag="xe_s")
                        nc.sync.dma_start(out=xt_[:], in_=d_x.ap()[c * 128:(c + 1) * 128, :])
                        wts = {}
                        for bi, b in enumerate(branches):
                            w_ = st.tile([128, G * R], wdt, tag=f"wp_{'ab'[bi]}")
                            nc.sync.dma_start(out=w_[:], in_=d_w[b].ap()[c * 128:(c + 1) * 128, :])
                            wts[b] = w_
                        if after_chunk is not None:
                            after_chunk(c)
                        for j in range(G):
                            for e in range(ET):
                                lhs = xt_[:, j * width + e * 128: j * width + (e + 1) * 128]
                                for b in branches:
                                    nc.tensor.matmul(
                                        ps[b][e][:], lhs, wts[b][:, j * R:(j + 1) * R],
                                        start=(c == 0 and j == 0),
                                        stop=(c == nchunks - 1 and j == G - 1))
                    return ps

                def evac(branch, ps, slot):
                    for e in range(ET):
                        tt = ts_.tile([128, R], B16, tag=f"T{slot}{e}")
                        nc.scalar.copy(tt[:], ps[e][:])
                        tstore[(branch, e)] = tt

                # ---- SDPA on DVE/ACT (all-bf16, tree reductions, stage-pipelined)
                x_tiles = [xa.tile([128, E], F32, tag=f"x{t}", name=f"x{t}") for t in range(NT)]

                def _tree_halve(view, width):
                    # in-place sum over the innermost axis via halving adds
                    w = width // 2
                    while w >= 1:
                        nc.vector.tensor_tensor(out=view[:, :, 0:w], in0=view[:, :, 0:w],
                                                in1=view[:, :, w:2 * w], op=ALU.add)
                        w //= 2

                def sdpa_branch(branch, first, after_tile=None):
                    Ps, Ebs = {}, {}

                    def stage_qk(t):
                        qb = qkv[(f"q_{branch}", t)]
                        kb = qkv[(f"k_{branch}", t)]
                        P = sp1.tile([128, H * H * D], B16, tag="P")
                        q_ap = qb[:].rearrange("p (h d) -> p h d", h=H).unsqueeze(2).broadcast_to((128, H, H, D))
                        k_ap = kb[:].rearrange("p (g d) -> p g d", g=H).unsqueeze(1).broadcast_to((128, H, H, D))
                        nc.vector.tensor_tensor(out=P[:].rearrange("p (h g d) -> p h g d", h=H, g=H),
                                                in0=q_ap, in1=k_ap, op=ALU.mult)
                        _tree_halve(P[:].rearrange("p (hg d) -> p hg d", hg=H * H), D)
                        Ps[t] = P

                    def stage_exp(t):
                        s_ap = Ps[t][:].rearrange("p (hg d) -> p hg d", hg=H * H)[:, :, 0:1] \
                            .rearrange("p hg one -> p (hg one)")
                        Eb = sp.tile([128, H * H], B16, tag="Eb")
                        nc.scalar.activation(Eb[:], s_ap, AF.Exp)
                        Ebs[t] = Eb

                    def stage_av(t):
                        Eb = Ebs.pop(t)
                        Ps.pop(t)
                        vb = qkv[(f"v_{branch}", t)]
                        den = sp.tile([128, H], F32, tag="den")
                        nc.vector.reduce_sum(out=den[:], in_=Eb[:].rearrange("p (h g) -> p h g", g=H),
                                             axis=AX.X)
                        rec = sp.tile([128, H], F32, tag="rec")
                        nc.vector.reciprocal(rec[:], den[:])
                        at = sp.tile([128, H * H], B16, tag="a")
                        nc.vector.tensor_tensor(
                            out=at[:].rearrange("p (h g) -> p h g", h=H),
                            in0=Eb[:].rearrange("p (h g) -> p h g", h=H),
                            in1=rec[:].unsqueeze(2).broadcast_to((128, H, H)),
                            op=ALU.mult)
                        Pa = sp1.tile([128, H * D * H], B16, tag="Pa")
                        a_ap = at[:].rearrange("p (h g) -> p h g", h=H).unsqueeze(2).broadcast_to((128, H, D, H))
                        v_ap = vb[:].rearrange("p (d g) -> p d g", g=H).unsqueeze(1).broadcast_to((128, H, D, H))
                        nc.vector.tensor_tensor(out=Pa[:].rearrange("p (h d g) -> p h d g", h=H, d=D),
                                                in0=a_ap, in1=v_ap, op=ALU.mult)
                        _tree_halve(Pa[:].rearrange("p (hd g) -> p hd g", hd=H * D), H)
                        av_ap = Pa[:].rearrange("p (hd g) -> p hd g", hd=H * D)[:, :, 0:1] \
                            .rearrange("p hd one -> p (hd one)")
                        xt = x_tiles[t]
                        if first:
                            nc.scalar.copy(xt[:], av_ap)
                        else:
                            nc.vector.tensor_tensor(out=xt[:], in0=xt[:], in1=av_ap, op=ALU.add)
                        if after_tile is not None:
                            after_tile(t)

                    stage_qk(0)
                    stage_exp(0)
                    for t in range(1, NT):
                        stage_qk(t)
                        stage_exp(t)
                        stage_av(t - 1)
                    stage_av(NT - 1)

                # ================= schedule =================
                # Pass order: hh -> (he+eh merged, shared xe stream) -> ee.
                # sdpa_hh overlaps the merged pass; sdpa_he/eh overlap the ee
                # pass; sdpa_ee + FFN form the tail.
                ps_hh = edge_pass(["hh"], {"hh": [0, 1, 2, 3]}, d_xnb,
                                  {"hh": d_adjt}, NC, E, wdt=F8,
                                  after_chunk=lambda c: load_residents() if c == 3 else None)["hh"]
                evac("hh", ps_hh, "A")

                proj_group([("k_hh", xnt_src, load_wn("k_hh")),
                            ("v_hh", xnt_src, load_wn("v_hh"))], banks=[4, 5])
                proj_group([("q_he", xnt_src, load_wn("q_he")),
                            ("v_he", xnt_src, load_wn("v_he"))], banks=[6, 7])
                proj_group([("v_ee", xnt_src, load_wn("v_ee")),
                            ("k_eh", xnt_src, load_wn("k_eh")),
                            ("v_eh", xnt_src, load_wn("v_eh"))], banks=[4, 5, 6])
                proj_group([("q_hh", ts_src("hh"), wt_ap("q_hh"))], banks=[7])
                sdpa_branch("hh", first=True)

                ps_m = edge_pass(["he", "eh"],
                                 {"he": [0, 1, 2, 3], "eh": [4, 5, 6, 7]},
                                 d_xe, {"he": d_wp["he"], "eh": d_wp["eh"]},
                                 MC, E)
                evac("he", ps_m["he"], "B")
                evac("eh", ps_m["eh"], "A")

                def mid_ee(c):
                    if c != 0:
                        return
                    proj_group([("k_he", ts_src("he"), wt_ap("k_he"))], banks=[0])
                    sdpa_branch("he", first=False)
                    proj_group([("q_eh", ts_src("eh"), wt_ap("q_eh"))], banks=[1])
                    sdpa_branch("eh", first=False)

                ps_ee = edge_pass(["ee"], {"ee": [4, 5, 6, 7]}, d_xe,
                                  {"ee": d_wp["ee"]}, MC, E,
                                  after_chunk=mid_ee)["ee"]
                evac("ee", ps_ee, "B")
                proj_group([("q_ee", ts_src("ee"), wt_ap("q_ee")),
                            ("k_ee", ts_src("ee"), wt_ap("k_ee"))], banks=[0, 1])

                # ================= FFN (pipelined) =================
                yT = [xa.tile([128, R], B16, tag=f"yT{e}", name=f"yT{e}") for e in range(ET)]

                def ffn_prep(t):
                    xt = x_tiles[t]
                    scr = sp.tile([128, E], B16, tag="scr")
                    nc.scalar.activation(scr[:], xt[:], AF.Square)
                    ms = mp.tile([128, 1], F32, tag=f"ms{t}")
                    nc.vector.reduce_sum(out=ms[:], in_=scr[:], axis=AX.X)
                    sd = mp.tile([128, 1], F32, tag=f"sd{t}")
                    nc.scalar.activation(sd[:], ms[:], AF.Sqrt, scale=1.0 / E, bias=eps_t[:])
                    inv2 = mp.tile([128, 1], F32, tag=f"inv{t}")
                    nc.vector.reciprocal(inv2[:], sd[:])
                    yt = sp.tile([128, E], B16, tag="y")
                    nc.vector.tensor_scalar_mul(yt[:], xt[:], inv2[:])
                    for e in range(ET):
                        pst = pp.tile([128, 128], B16, tag=f"bank{6 + (e % 2)}")
                        nc.tensor.transpose(pst[:], yt[:, e * 128:(e + 1) * 128], ident[:])
                        nc.scalar.copy(yT[e][:, t * 128:(t + 1) * 128], pst[:])

                b1 = mp.tile([128, FT], F32, tag="b1")
                nc.sync.dma_start(out=b1[:], in_=d_b1t.ap())
                ones_t = mp.tile([1, 128], B16, tag="ones")
                nc.sync.dma_start(out=ones_t[:], in_=d_ones.ap())
                wtb = qs.tile([128, E], B16, tag="wtb")
                nc.sync.dma_start(out=wtb[:], in_=d_wtb.ap())

                pso = [pp.tile([128, E], F32, tag=f"bank{4 + t}", name=f"pso{t}")
                       for t in range(NT)]
                zts = {}

                def ffn1(f):
                    w1 = ws.tile([128, ET * 128], B16, tag="wf1s")
                    nc.sync.dma_start(out=w1[:], in_=d_wf1.ap()[f * 128:(f + 1) * 128, :])
                    psz = pp.tile([128, R], F32, tag=f"bank{f % 3}")
                    for k in range(ET):
                        nc.tensor.matmul(psz[:], w1[:, k * 128:(k + 1) * 128], yT[k][:],
                                         start=(k == 0), stop=(k == ET - 1))
                    zt = zs.tile([128, R], B16, tag=f"zT{f % 3}")
                    nc.scalar.activation(zt[:], psz[:],
                                         AF.Identity if sim_safe else AF.Gelu,
                                         bias=b1[:, f:f + 1])
                    zts[f] = zt

                def ffn2(f):
                    w2 = ws.tile([128, E], B16, tag="wf2s")
                    nc.sync.dma_start(out=w2[:], in_=d_wf2.ap()[f * 128:(f + 1) * 128, :])
                    zt = zts.pop(f)
                    last = (f == FT - 1)
                    for t in range(NT):
                        nc.tensor.matmul(pso[t][:], zt[:, t * 128:(t + 1) * 128], w2[:],
                                         start=(f == 0), stop=False)
                        if last:
                            nc.tensor.matmul(pso[t][:], ones_t[0:1, :], wtb[0:1, :],
                                             start=False, stop=True)
                            ot = sp.tile([128, E], F32, tag="ot")
                            if t % 2 == 0:
                                nc.scalar.copy(ot[:], pso[t][:])
                            else:
                                nc.vector.tensor_scalar_mul(ot[:], pso[t][:], 1.0)
                            nc.sync.dma_start(out=d_out.ap()[t * 128:(t + 1) * 128, :],
                                              in_=ot[:])

                def after_tile_ee(t):
                    ffn_prep(t)
                    if t == NT - 1:
                        ffn1(0)
                        ffn1(1)
                        for f in range(2, FT):
                            ffn1(f)
                            ffn2(f - 2)
                        ffn2(FT - 2)
                        ffn2(FT - 1)

                sdpa_branch("ee", first=False, after_tile=after_tile_ee)

            if loopn > 1:
                with tc.For_i(0, loopn, 1) as _i:
                    body(_i)
            else:
                body()

    nc.compile()
    return nc


def _chunk_rows(arr, G):
    """(T*128, W) -> (T/G * 128, G*W): m-tiles batched G per 128-partition chunk."""
    T128, W = arr.shape
    T = T128 // 128
    C = T // G
    return np.ascontiguousarray(
        arr.reshape(C, G, 128, W).transpose(0, 2, 1, 3)).reshape(C * 128, G * W)


def _kmajor(arr, KT):
    """(KT*128, W) -> (128, KT*W): k-tiles side by side per partition."""
    _, W = arr.shape
    return np.ascontiguousarray(
        arr.reshape(KT, 128, W).transpose(1, 0, 2)).reshape(128, KT * W)


def _prep_inputs(inputs, scale=1):
    """Host-side folding + sharding. Returns per-core in_maps."""
    dm = _dims(scale)
    N, M, E, R, F, FT, ET = dm["N"], dm["M"], dm["E"], dm["R"], dm["F"], dm["FT"], dm["ET"]
    x_node = np.asarray(inputs["x_node"], np.float32)
    x_edge = np.asarray(inputs["x_edge"], np.float32)
    adj = np.asarray(inputs["adj"], np.float32)
    g_n = np.asarray(inputs["g_n"], np.float32)
    g_e = np.asarray(inputs["g_e"], np.float32)
    g2 = np.asarray(inputs["g2"], np.float32)

    inv_n = (1.0 / np.sqrt((x_node.astype(np.float64) ** 2).mean(axis=1) + 1e-6)).astype(np.float32)
    inv_e = (1.0 / np.sqrt((x_edge.astype(np.float64) ** 2).mean(axis=1) + 1e-6)).astype(np.float32)
    xn_s = x_node * inv_n[:, None]
    xe_s = x_edge * inv_e[:, None]

    perm = np.array([(j % H) * D + j // H for j in range(E)])  # newcol j=(d,g) <- oldcol g*D+d

    def fold_q(w, g):
        return (g[:, None] * np.asarray(w, np.float32)) / np.sqrt(D)

    def fold_k(w, g):
        return g[:, None] * np.asarray(w, np.float32)

    def fold_v(w, g):
        return (g[:, None] * np.asarray(w, np.float32))[:, perm]

    wt = {
        "q_hh": fold_q(inputs["Wq_hh"], g_n),
        "q_ee": fold_q(inputs["Wq_ee"], g_e),
        "k_ee": fold_k(inputs["Wk_ee"], g_e),
        "q_eh": fold_q(inputs["Wq_eh"], g_e),
        "k_he": fold_k(inputs["Wk_he"], g_e),
    }
    wn = {
        "k_hh": fold_k(inputs["Wk_hh"], g_n),
        "v_hh": fold_v(inputs["Wv_hh"], g_n),
        "v_ee": fold_v(inputs["Wv_ee"], g_n),
        "k_eh": fold_k(inputs["Wk_eh"], g_n),
        "v_eh": fold_v(inputs["Wv_eh"], g_n),
        "q_he": fold_q(inputs["Wq_he"], g_n),
        "v_he": fold_v(inputs["Wv_he"], g_n),
    }
    wf1 = g2[:, None] * np.asarray(inputs["Wf1"], np.float32)
    bf1 = np.asarray(inputs["bf1"], np.float32)
    b1t = np.ascontiguousarray(bf1.reshape(FT, 128).T)
    # wf1 layout: [f*128+p, k*128+c] = wf1[k*128+p, f*128+c]
    wf1c = np.ascontiguousarray(
        wf1.reshape(ET, 128, FT, 128).transpose(2, 1, 0, 3)).reshape(FT * 128, ET * 128)
    wtb = np.zeros((128, E), np.float32)
    wtb[0] = np.asarray(inputs["bf2"], np.float32)
    wtb = wtb.astype(BF16)

    shared = {
        "xe": _chunk_rows(xe_s.astype(BF16), G),
        "xnb": _chunk_rows(xn_s.astype(BF16), G),
        "b1t": b1t,
        "wf1": wf1c.astype(BF16),
        "wf2": np.ascontiguousarray(np.asarray(inputs["Wf2"], np.float32)).astype(BF16),
        "wtb": wtb,
        "ident": np.eye(128, dtype=np.float32).astype(BF16),
        "onesrow": np.ones((1, 128), np.float32).astype(BF16),
    }
    for k, v in wt.items():
        shared[f"w_{k}"] = _kmajor(v.astype(BF16), ET)
    for k, v in wn.items():
        shared[f"w_{k}"] = _kmajor(v.astype(BF16), ET)

    wp1 = {b: np.asarray(inputs[f"Wp1_{b}"], np.float32) for b in ("ee", "eh", "he")}
    in_maps = []
    for c in range(NCORES):
        rows = slice(c * R, (c + 1) * R)
        m = dict(shared)
        m["adjt"] = _chunk_rows(
            np.ascontiguousarray(adj[rows].T).astype(ml_dtypes.float8_e4m3), G)
        m["xnt"] = _kmajor(np.ascontiguousarray(xn_s[rows].T).astype(BF16), ET)
        for b in ("ee", "eh", "he"):
            m[f"wp1t_{b}"] = _chunk_rows(
                np.ascontiguousarray(wp1[b][rows].T).astype(BF16), G)
        in_maps.append(m)
    return in_maps


def kernel(**inputs) -> np.ndarray:
    from concourse.bass_utils import run_bass_kernel_spmd

    if "nc" not in _CACHE:
        _CACHE["nc"] = _build()
    nc = _CACHE["nc"]
    in_maps = _prep_inputs(inputs)
    res = run_bass_kernel_spmd(nc, in_maps, list(range(NCORES)))
    out = np.concatenate([res.results[c]["out"] for c in range(NCORES)], axis=0)
    return np.ascontiguousarray(out, dtype=np.float32)


# revision 31
# speedup vs baseline: 1.0028x; 1.0028x over previous
"""Trainium2 Bass kernel for nn_Block_7584912244953 (gnn_message_passing).

Strategy (8 NeuronCores, SPMD, node-row sharding, no collectives):
  - Associativity: Wp1 @ (e @ W) == (Wp1 @ e) @ W. Each core computes
    T_b^T = e^T @ Wp1_b[rows]^T for its 512 node rows (contraction over
    all 16384 edges), then small (512x512) projections.
  - RMS norms folded on host; gains and 1/sqrt(D) folded into weights.
  - Schedule: T_hh pass -> node-side projections -> T_he pass (sdpa_hh
    on DVE underneath) -> T_eh pass (sdpa_he underneath) -> T_ee pass
    (sdpa_eh underneath) -> sdpa_ee + FFN (software-pipelined).
  - Stream DMAs batched 4 m-tiles per transfer (HWDGE issue rate is
    ~625ns/DMA; fewer, bigger transfers).
  - SDPA on DVE with broadcast APs, all-bf16 operands (2x mode); exp
    values scaled by 1/den before the AV product so no final rescale.
"""

import numpy as np
import ml_dtypes

BF16 = ml_dtypes.bfloat16
NCORES = 8
H, D = 8, 64
G = 4  # m-tiles per DMA chunk
_CACHE = {}


def _dims(scale=1):
    N, M, E = 4096 // scale, 16384 // scale, 512
    R = N // NCORES
    return dict(N=N, M=M, E=E, R=R, NT=R // 128, ET=E // 128, MT=M // 128,
                NMT=N // 128, F=4 * E, FT=4 * E // 128,
                MC=M // 128 // G, NC=N // 128 // G)


WT_B = ["q_hh", "q_ee", "k_ee", "q_eh", "k_he"]
WN_B = ["k_hh", "v_hh", "v_ee", "k_eh", "v_eh", "q_he", "v_he"]


def _build(scale=1, loopn=1, sim_safe=False):
    import concourse.bacc as bacc
    import concourse.mybir as mybir
    from concourse import tile

    dm = _dims(scale)
    N, M, E, R = dm["N"], dm["M"], dm["E"], dm["R"]
    NT, ET, F, FT = dm["NT"], dm["ET"], dm["F"], dm["FT"]
    MC, NC = dm["MC"], dm["NC"]

    F32 = mybir.dt.float32
    F32R = mybir.dt.float32r
    B16 = mybir.dt.bfloat16
    F8 = mybir.dt.float8e4
    AF = mybir.ActivationFunctionType
    ALU = mybir.AluOpType
    AX = mybir.AxisListType

    nc = bacc.Bacc("TRN2", target_bir_lowering=False, debug=False, num_devices=NCORES)

    d_xe = nc.dram_tensor("xe", [MC * 128, G * E], B16, kind="ExternalInput")
    d_wp = {b: nc.dram_tensor(f"wp1t_{b}", [MC * 128, G * R], B16, kind="ExternalInput")
            for b in ("ee", "eh", "he")}
    d_adjt = nc.dram_tensor("adjt", [NC * 128, G * R], F8, kind="ExternalInput")
    d_xnb = nc.dram_tensor("xnb", [NC * 128, G * E], B16, kind="ExternalInput")
    d_xnt = nc.dram_tensor("xnt", [128, ET * R], B16, kind="ExternalInput")
    d_wt = {w: nc.dram_tensor(f"w_{w}", [128, ET * E], B16, kind="ExternalInput") for w in WT_B}
    d_wn = {w: nc.dram_tensor(f"w_{w}", [128, ET * E], B16, kind="ExternalInput") for w in WN_B}
    d_wf1 = nc.dram_tensor("wf1", [FT * 128, ET * 128], B16, kind="ExternalInput")
    d_b1t = nc.dram_tensor("b1t", [128, FT], F32, kind="ExternalInput")
    d_wf2 = nc.dram_tensor("wf2", [F, E], B16, kind="ExternalInput")
    d_wtb = nc.dram_tensor("wtb", [128, E], B16, kind="ExternalInput")
    d_id = nc.dram_tensor("ident", [128, 128], B16, kind="ExternalInput")
    d_ones = nc.dram_tensor("onesrow", [1, 128], B16, kind="ExternalInput")
    d_out = nc.dram_tensor("out", [R, E], F32, kind="ExternalOutput")

    with tile.TileContext(nc) as tc:
        with (
            tc.tile_pool(name="stream", bufs=4) as st,
            tc.tile_pool(name="xnts", bufs=1) as xs,
            tc.tile_pool(name="wtres", bufs=1) as wtp,
            tc.tile_pool(name="wnres", bufs=1) as wn_,
            tc.tile_pool(name="tstore", bufs=1) as ts_,
            tc.tile_pool(name="qkv", bufs=1) as qs,
            tc.tile_pool(name="sdpa1", bufs=2) as sp1,
            tc.tile_pool(name="sdpa2", bufs=1) as sp2,
            tc.tile_pool(name="sdpa", bufs=2) as sp,
            tc.tile_pool(name="xacc", bufs=1) as xa,
            tc.tile_pool(name="zst", bufs=2) as zs,
            tc.tile_pool(name="wstream", bufs=2) as ws,
            tc.tile_pool(name="misc", bufs=1) as mp,
            tc.tile_pool(name="psum", bufs=1, space="PSUM") as pp,
        ):
            def body(iv=None):
                qkv = {}     # (name, t) -> sbuf bf16 tile (128, 512)
                tstore = {}  # (branch, e) -> T^T tile bf16 (128, R)

                # ---- residents (loaded lazily, after the first stream chunks)
                xnt_c = xs.tile([128, ET * R], B16, tag="xntc")
                ident = mp.tile([128, 128], B16, tag="ident")
                eps_t = mp.tile([128, 1], F32, tag="eps")

                def load_residents():
                    nc.sync.dma_start(out=xnt_c[:], in_=d_xnt.ap())
                    nc.sync.dma_start(out=ident[:], in_=d_id.ap())
                    nc.gpsimd.memset(eps_t[:], 1e-6)

                _wn_slot = [0]

                def load_wn(name):
                    s = _wn_slot[0] = (_wn_slot[0] + 1) % 3
                    t_ = wn_.tile([128, ET * E], B16, tag=f"wn_{s}")
                    nc.sync.dma_start(out=t_[:], in_=d_wn[name].ap())
                    return lambda k: t_[:, k * E:(k + 1) * E]

                _wt_slot = [0]

                def wt_ap(name):
                    s = _wt_slot[0] = (_wt_slot[0] + 1) % 2
                    t_ = wtp.tile([128, ET * E], B16, tag=f"wt_{s}", name=f"wt_{s}")
                    nc.sync.dma_start(out=t_[:], in_=d_wt[name].ap())
                    return lambda k: t_[:, k * E:(k + 1) * E]

                def xnt_src(k, t):
                    return xnt_c[:, k * R + t * 128: k * R + (t + 1) * 128]

                def ts_src(branch):
                    return lambda k, t: tstore[(branch, k)][:, t * 128:(t + 1) * 128]

                # ---- grouped projections: (name, src_fn(k,t), wt_fn(k))
                def proj_group(specs, banks):
                    for t in range(NT):
                        psb = [pp.tile([128, E], F32, tag=f"bank{banks[i]}",
                                       name=f"psb{banks[i]}")
                               for i in range(len(specs))]
                        for k in range(ET):
                            for i, (name, src, wtf) in enumerate(specs):
                                nc.tensor.matmul(psb[i][:], src(k, t), wtf(k),
                                                 start=(k == 0), stop=(k == ET - 1))
                        for i, (name, src, wtf) in enumerate(specs):
                            q = qs.tile([128, E], B16, tag=f"{name}{t}", name=f"{name}{t}")
                            nc.scalar.copy(q[:], psb[i][:])
                            qkv[(name, t)] = q

                # ---- T passes: contraction over chunked streams
                def edge_pass(branches, banks, d_x, d_w, nchunks, width,
                                              wdt=B16, after_chunk=None):
                    ps = {b: [pp.tile([128, R], F32, tag=f"bank{banks[b][e]}",
                                      name=f"ps_{b}{e}")
                              for e in range(ET)] for b in branches}
                    for c in range(nchunks):
                        xt_ = st.tile([128, G * width], B16, tag="xe_s")
                        nc.sync.dma_start(out=xt_[:], in_=d_x.ap()[c * 128:(c + 1) * 128, :])
                        wts = {}
                        for bi, b in enumerate(branches):
                            w_ = st.tile([128, G * R], wdt, tag=f"wp_{'ab'[bi]}")
                            nc.sync.dma_start(out=w_[:], in_=d_w[b].ap()[c * 128:(c + 1) * 128, :])
                            wts[b] = w_
                        if after_chunk is not None:
                            after_chunk(c)
                        for j in range(G):
                            for e in range(ET):
                                lhs = xt_[:, j * width + e * 128: j * width + (e + 1) * 128]
                                for b in branches:
                                    nc.tensor.matmul(
                                        ps[b][e][:], lhs, wts[b][:, j * R:(j + 1) * R],
                                        start=(c == 0 and j == 0),
                                        stop=(c == nchunks - 1 and j == G - 1))
                    return ps

                def evac(branch, ps, slot):
                    for e in range(ET):
                        tt = ts_.tile([128, R], B16, tag=f"T{slot}{e}")
                        nc.scalar.copy(tt[:], ps[e][:])
                        tstore[(branch, e)] = tt

                # ---- SDPA on DVE/ACT (all-bf16, tree reductions, stage-pipelined)
                x_tiles = [xa.tile([128, E], F32, tag=f"x{t}", name=f"x{t}") for t in range(NT)]

                def _tree_halve(view, width):
                    # in-place sum over the innermost axis via halving adds
                    w = width // 2
                    while w >= 1:
                        nc.vector.tensor_tensor(out=view[:, :, 0:w], in0=view[:, :, 0:w],
                                                in1=view[:, :, w:2 * w], op=ALU.add)
                        w //= 2

                def sdpa_branch(branch, first, after_tile=None):
                    Ps, Ebs = {}, {}

                    def stage_qk(t):
                        qb = qkv[(f"q_{branch}", t)]
                        kb = qkv[(f"k_{branch}", t)]
                        P = sp1.tile([128, H * H * D], B16, tag="P")
                        q_ap = qb[:].rearrange("p (h d) -> p h d", h=H).unsqueeze(2).broadcast_to((128, H, H, D))
                        k_ap = kb[:].rearrange("p (g d) -> p g d", g=H).unsqueeze(1).broadcast_to((128, H, H, D))
                        nc.vector.tensor_tensor(out=P[:].rearrange("p (h g d) -> p h g d", h=H, g=H),
                                                in0=q_ap, in1=k_ap, op=ALU.mult)
                        _tree_halve(P[:].rearrange("p (hg d) -> p hg d", hg=H * H), D)
                        Ps[t] = P

                    def stage_exp(t):
                        s_ap = Ps[t][:].rearrange("p (hg d) -> p hg d", hg=H * H)[:, :, 0:1] \
                            .rearrange("p hg one -> p (hg one)")
                        Eb = sp.tile([128, H * H], B16, tag="Eb")
                        nc.scalar.activation(Eb[:], s_ap, AF.Exp)
                        Ebs[t] = Eb

                    def stage_av(t):
                        Eb = Ebs.pop(t)
                        Ps.pop(t)
                        vb = qkv[(f"v_{branch}", t)]
                        den = sp.tile([128, H], F32, tag="den")
                        nc.vector.reduce_sum(out=den[:], in_=Eb[:].rearrange("p (h g) -> p h g", g=H),
                                             axis=AX.X)
                        rec = sp.tile([128, H], F32, tag="rec")
                        nc.vector.reciprocal(rec[:], den[:])
                        at = sp.tile([128, H * H], B16, tag="a")
                        nc.vector.tensor_tensor(
                            out=at[:].rearrange("p (h g) -> p h g", h=H),
                            in0=Eb[:].rearrange("p (h g) -> p h g", h=H),
                            in1=rec[:].unsqueeze(2).broadcast_to((128, H, H)),
                            op=ALU.mult)
                        Pa = sp1.tile([128, H * D * H], B16, tag="Pa")
                        a_ap = at[:].rearrange("p (h g) -> p h g", h=H).unsqueeze(2).broadcast_to((128, H, D, H))
                        v_ap = vb[:].rearrange("p (d g) -> p d g", g=H).unsqueeze(1).broadcast_to((128, H, D, H))
                        nc.vector.tensor_tensor(out=Pa[:].rearrange("p (h d g) -> p h d g", h=H, d=D),
                                                in0=a_ap, in1=v_ap, op=ALU.mult)
                        _tree_halve(Pa[:].rearrange("p (hd g) -> p hd g", hd=H * D), H)
                        av_ap = Pa[:].rearrange("p (hd g) -> p hd g", hd=H * D)[:, :, 0:1] \
                            .rearrange("p hd one -> p (hd one)")
                        xt = x_tiles[t]
                        if first:
                            nc.scalar.copy(xt[:], av_ap)
                        else:
                            nc.vector.tensor_tensor(out=xt[:], in0=xt[:], in1=av_ap, op=ALU.add)
                        if after_tile is not None:
                            after_tile(t)

                    stage_qk(0)
                    stage_exp(0)
                    for t in range(1, NT):
                        stage_qk(t)
                        stage_exp(t)
                        stage_av(t - 1)
                    stage_av(NT - 1)

                # ================= schedule =================
                # Pass order: hh -> (he+eh merged, shared xe stream) -> ee.
                # sdpa_hh overlaps the merged pass; sdpa_he/eh overlap the ee
                # pass; sdpa_ee + FFN form the tail.
                ps_hh = edge_pass(["hh"], {"hh": [0, 1, 2, 3]}, d_xnb,
                                  {"hh": d_adjt}, NC, E, wdt=F8,
                                  after_chunk=lambda c: load_residents() if c == 3 else None)["hh"]
                evac("hh", ps_hh, "A")

                proj_group([("k_hh", xnt_src, load_wn("k_hh")),
                            ("v_hh", xnt_src, load_wn("v_hh"))], banks=[4, 5])
                proj_group([("q_he", xnt_src, load_wn("q_he")),
                            ("v_he", xnt_src, load_wn("v_he"))], banks=[6, 7])
                proj_group([("v_ee", xnt_src, load_wn("v_ee")),
                            ("k_eh", xnt_src, load_wn("k_eh")),
                            ("v_eh", xnt_src, load_wn("v_eh"))], banks=[4, 5, 6])
                proj_group([("q_hh", ts_src("hh"), wt_ap("q_hh"))], banks=[7])
                sdpa_branch("hh", first=True)

                ps_m = edge_pass(["he", "eh"],
                                 {"he": [0, 1, 2, 3], "eh": [4, 5, 6, 7]},
                                 d_xe, {"he": d_wp["he"], "eh": d_wp["eh"]},
                                 MC, E)
                evac("he", ps_m["he"], "B")
                evac("eh", ps_m["eh"], "A")

                def mid_ee(c):
                    if c != 1:
                        return
                    proj_group([("k_he", ts_src("he"), wt_ap("k_he"))], banks=[0])
                    sdpa_branch("he", first=False)
                    proj_group([("q_eh", ts_src("eh"), wt_ap("q_eh"))], banks=[1])
                    sdpa_branch("eh", first=False)

                ps_ee = edge_pass(["ee"], {"ee": [4, 5, 6, 7]}, d_xe,
                                  {"ee": d_wp["ee"]}, MC, E,
                                  after_chunk=mid_ee)["ee"]
                evac("ee", ps_ee, "B")
                proj_group([("q_ee", ts_src("ee"), wt_ap("q_ee")),
                            ("k_ee", ts_src("ee"), wt_ap("k_ee"))], banks=[0, 1])

                # ================= FFN (pipelined) =================
                yT = [xa.tile([128, R], B16, tag=f"yT{e}", name=f"yT{e}") for e in range(ET)]

                def ffn_prep(t):
                    xt = x_tiles[t]
                    scr = sp.tile([128, E], B16, tag="scr")
                    nc.scalar.activation(scr[:], xt[:], AF.Square)
                    ms = mp.tile([128, 1], F32, tag=f"ms{t}")
                    nc.vector.reduce_sum(out=ms[:], in_=scr[:], axis=AX.X)
                    sd = mp.tile([128, 1], F32, tag=f"sd{t}")
                    nc.scalar.activation(sd[:], ms[:], AF.Sqrt, scale=1.0 / E, bias=eps_t[:])
                    inv2 = mp.tile([128, 1], F32, tag=f"inv{t}")
                    nc.vector.reciprocal(inv2[:], sd[:])
                    yt = sp.tile([128, E], B16, tag="y")
                    nc.vector.tensor_scalar_mul(yt[:], xt[:], inv2[:])
                    for e in range(ET):
                        pst = pp.tile([128, 128], B16, tag=f"bank{6 + (e % 2)}")
                        nc.tensor.transpose(pst[:], yt[:, e * 128:(e + 1) * 128], ident[:])
                        nc.scalar.copy(yT[e][:, t * 128:(t + 1) * 128], pst[:])

                b1 = mp.tile([128, FT], F32, tag="b1")
                nc.sync.dma_start(out=b1[:], in_=d_b1t.ap())
                ones_t = mp.tile([1, 128], B16, tag="ones")
                nc.sync.dma_start(out=ones_t[:], in_=d_ones.ap())
                wtb = qs.tile([128, E], B16, tag="wtb")
                nc.sync.dma_start(out=wtb[:], in_=d_wtb.ap())

                pso = [pp.tile([128, E], F32, tag=f"bank{4 + t}", name=f"pso{t}")
                       for t in range(NT)]
                zts = {}

                def ffn1(f):
                    w1 = ws.tile([128, ET * 128], B16, tag="wf1s")
                    nc.sync.dma_start(out=w1[:], in_=d_wf1.ap()[f * 128:(f + 1) * 128, :])
                    psz = pp.tile([128, R], F32, tag=f"bank{f % 3}")
                    for k in range(ET):
                        nc.tensor.matmul(psz[:], w1[:, k * 128:(k + 1) * 128], yT[k][:],
                                         start=(k == 0), stop=(k == ET - 1))
                    zt = zs.tile([128, R], B16, tag=f"zT{f % 3}")
                    nc.scalar.activation(zt[:], psz[:],
                                         AF.Identity if sim_safe else AF.Gelu,
                                         bias=b1[:, f:f + 1])
                    zts[f] = zt

                def ffn2(f):
                    w2 = ws.tile([128, E], B16, tag="wf2s")
                    nc.sync.dma_start(out=w2[:], in_=d_wf2.ap()[f * 128:(f + 1) * 128, :])
                    zt = zts.pop(f)
                    last = (f == FT - 1)
                    for t in range(NT):
                        nc.tensor.matmul(pso[t][:], zt[:, t * 128:(t + 1) * 128], w2[:],
                                         start=(f == 0), stop=False)
                        if last:
                            nc.tensor.matmul(pso[t][:], ones_t[0:1, :], wtb[0:1, :],
                                             start=False, stop=True)
                            ot = sp.tile([128, E], F32, tag="ot")
                            if t % 2 == 0:
                                nc.scalar.copy(ot[:], pso[t][:])
                            else:
                                nc.vector.tensor_scalar_mul(ot[:], pso[t][:], 1.0)
                            nc.sync.dma_start(out=d_out.ap()[t * 128:(t + 1) * 128, :],
                                              in_=ot[:])

                def after_tile_ee(t):
                    ffn_prep(t)
                    if t == NT - 1:
                        ffn1(0)
                        ffn1(1)
                        for f in range(2, FT):
                            ffn1(f)
                            ffn2(f - 2)
                        ffn2(FT - 2)
                        ffn2(FT - 1)

                sdpa_branch("ee", first=False, after_tile=after_tile_ee)

            if loopn > 1:
                with tc.For_i(0, loopn, 1) as _i:
                    body(_i)
            else:
                body()

    nc.compile()
    return nc


def _chunk_rows(arr, G):
    """(T*128, W) -> (T/G * 128, G*W): m-tiles batched G per 128-partition chunk."""
    T128, W = arr.shape
    T = T128 // 128
    C = T // G
    return np.ascontiguousarray(
        arr.reshape(C, G, 128, W).transpose(0, 2, 1, 3)).reshape(C * 128, G * W)


def _kmajor(arr, KT):
    """(KT*128, W) -> (128, KT*W): k-tiles side by side per partition."""
    _, W = arr.shape
    return np.ascontiguousarray(
        arr.reshape(KT, 128, W).transpose(1, 0, 2)).reshape(128, KT * W)


def _prep_inputs(inputs, scale=1):
    """Host-side folding + sharding. Returns per-core in_maps."""
    dm = _dims(scale)
    N, M, E, R, F, FT, ET = dm["N"], dm["M"], dm["E"], dm["R"], dm["F"], dm["FT"], dm["ET"]
    x_node = np.asarray(inputs["x_node"], np.float32)
    x_edge = np.asarray(inputs["x_edge"], np.float32)
    adj = np.asarray(inputs["adj"], np.float32)
    g_n = np.asarray(inputs["g_n"], np.float32)
    g_e = np.asarray(inputs["g_e"], np.float32)
    g2 = np.asarray(inputs["g2"], np.float32)

    inv_n = (1.0 / np.sqrt((x_node.astype(np.float64) ** 2).mean(axis=1) + 1e-6)).astype(np.float32)
    inv_e = (1.0 / np.sqrt((x_edge.astype(np.float64) ** 2).mean(axis=1) + 1e-6)).astype(np.float32)
    xn_s = x_node * inv_n[:, None]
    xe_s = x_edge * inv_e[:, None]

    perm = np.array([(j % H) * D + j // H for j in range(E)])  # newcol j=(d,g) <- oldcol g*D+d

    def fold_q(w, g):
        return (g[:, None] * np.asarray(w, np.float32)) / np.sqrt(D)

    def fold_k(w, g):
        return g[:, None] * np.asarray(w, np.float32)

    def fold_v(w, g):
        return (g[:, None] * np.asarray(w, np.float32))[:, perm]

    wt = {
        "q_hh": fold_q(inputs["Wq_hh"], g_n),
        "q_ee": fold_q(inputs["Wq_ee"], g_e),
        "k_ee": fold_k(inputs["Wk_ee"], g_e),
        "q_eh": fold_q(inputs["Wq_eh"], g_e),
        "k_he": fold_k(inputs["Wk_he"], g_e),
    }
    wn = {
        "k_hh": fold_k(inputs["Wk_hh"], g_n),
        "v_hh": fold_v(inputs["Wv_hh"], g_n),
        "v_ee": fold_v(inputs["Wv_ee"], g_n),
        "k_eh": fold_k(inputs["Wk_eh"], g_n),
        "v_eh": fold_v(inputs["Wv_eh"], g_n),
        "q_he": fold_q(inputs["Wq_he"], g_n),
        "v_he": fold_v(inputs["Wv_he"], g_n),
    }
    wf1 = g2[:, None] * np.asarray(inputs["Wf1"], np.float32)
    bf1 = np.asarray(inputs["bf1"], np.float32)
    b1t = np.ascontiguousarray(bf1.reshape(FT, 128).T)
    # wf1 layout: [f*128+p, k*128+c] = wf1[k*128+p, f*128+c]
    wf1c = np.ascontiguousarray(
        wf1.reshape(ET, 128, FT, 128).transpose(2, 1, 0, 3)).reshape(FT * 128, ET * 128)
    wtb = np.zeros((128, E), np.float32)
    wtb[0] = np.asarray(inputs["bf2"], np.float32)
    wtb = wtb.astype(BF16)

    shared = {
        "xe": _chunk_rows(xe_s.astype(BF16), G),
        "xnb": _chunk_rows(xn_s.astype(BF16), G),
        "b1t": b1t,
        "wf1": wf1c.astype(BF16),
        "wf2": np.ascontiguousarray(np.asarray(inputs["Wf2"], np.float32)).astype(BF16),
        "wtb": wtb,
        "ident": np.eye(128, dtype=np.float32).astype(BF16),
        "onesrow": np.ones((1, 128), np.float32).astype(BF16),
    }
    for k, v in wt.items():
        shared[f"w_{k}"] = _kmajor(v.astype(BF16), ET)
    for k, v in wn.items():
        shared[f"w_{k}"] = _kmajor(v.astype(BF16), ET)

    wp1 = {b: np.asarray(inputs[f"Wp1_{b}"], np.float32) for b in ("ee", "eh", "he")}
    in_maps = []
    for c in range(NCORES):
        rows = slice(c * R, (c + 1) * R)
        m = dict(shared)
        m["adjt"] = _chunk_rows(
            np.ascontiguousarray(adj[rows].T).astype(ml_dtypes.float8_e4m3), G)
        m["xnt"] = _kmajor(np.ascontiguousarray(xn_s[rows].T).astype(BF16), ET)
        for b in ("ee", "eh", "he"):
            m[f"wp1t_{b}"] = _chunk_rows(
                np.ascontiguousarray(wp1[b][rows].T).astype(BF16), G)
        in_maps.append(m)
    return in_maps


def kernel(**inputs) -> np.ndarray:
    from concourse.bass_utils import run_bass_kernel_spmd

    if "nc" not in _CACHE:
        _CACHE["nc"] = _build()
    nc = _CACHE["nc"]
    in_maps = _prep_inputs(inputs)
    res = run_bass_kernel_spmd(nc, in_maps, list(range(NCORES)))
    out = np.concatenate([res.results[c]["out"] for c in range(NCORES)], axis=0)
    return np.ascontiguousarray(out, dtype=np.float32)
